# revision 11
# baseline (speedup 1.0000x reference)
"""Bass/Trainium2 kernel for nn_Encoder_47210280517649 (Pyraformer encoder).

Strategy: pure data-parallel over batch (B=16 -> 2 per core across 8 cores).
Feature-major activations x^T [d_model(4x128 part), 680 tok(free)], fp32r
matmuls, CSCM/BatchNorm computed redundantly over all 16 batches on every
core (keeps BN exact without collectives), attention computed as S^T per
key-tile with softmax denominators obtained from a ones-column appended to V
and normalization broadcast via an indicator matmul.
"""
import sys
import numpy as np

try:
    import concourse  # noqa: F401
except ImportError:
    for _p in ("/opt/trn_rl_repo", "/root/.axon_site/_ro/trn_rl_repo"):
        sys.path.insert(0, _p)

import concourse.bass as bass  # noqa: E402
import concourse.mybir as mybir  # noqa: E402
import concourse.tile as tile  # noqa: E402
from concourse import bacc  # noqa: E402
from concourse.bass_utils import run_bass_kernel_spmd  # noqa: E402
from concourse.alu_op_type import AluOpType  # noqa: E402

dt = mybir.dt
F32, F32R = dt.float32, dt.float32r
Act = mybir.ActivationFunctionType

N_CORES = 8
NB = 16          # total batch
LB = 2           # local batch per core
SEQ = 512
DM = 512         # d_model
L = 680          # 512+128+32+8
NH = 8
DKH = 64
DFF = 2048
NL = 4
HW = 340         # query-half width
KT = [(0, 128), (128, 256), (256, 384), (384, 512), (512, 640), (640, 680)]
NEG = -10000.0

# token tiles for V (token-major)
VT = KT


def _mask_bias(attn_mask: np.ndarray) -> np.ndarray:
    return np.where(attn_mask, np.float32(NEG), np.float32(0.0))


def _nonempty(attn_mask):
    """(r, half) pairs that have at least one kept (non-blocked) entry."""
    keep = ~attn_mask
    live = {}
    for r, (a, b) in enumerate(KT):
        for h in range(2):
            live[(r, h)] = bool(keep[a:b, h * HW:(h + 1) * HW].any())
    return live


def _build_program(live):
    nc = bacc.Bacc("TRN2", target_bir_lowering=False, debug=False,
                   num_devices=N_CORES)

    def din(name, shape, dtp=F32R):
        return nc.dram_tensor(name, shape, dtp, kind="ExternalInput")

    # ---- DRAM inputs ----
    x25t = din("x25t", [NB, 25, SEQ])                 # folded conv+mark input
    w25 = din("w25", [25, DM])
    cembT = din("cembT", [4, 128, SEQ], F32)          # pos+mark_b, feat-major
    downw = din("downw", [4, 128, 128])
    convk = din("convk", [3, 4, 128, 128])
    upw = din("upw", [128, DM])
    maskb = din("maskb", [6, 128, L], F32)            # S^T additive mask tiles
    e8 = din("e8", [8, DM])                           # head indicator
    # weights per layer
    wq = din("wq", [NL, DM, DM]); wk = din("wk", [NL, DM, DM])
    wv = din("wv", [NL, DM, DM]); wo = din("wo", [NL, DM, DM])
    w1 = din("w1", [NL, DM, DFF]); w2 = din("w2", [NL, DFF, DM])
    bvB = din("bvB", [NL, 128, DM], F32)              # V bias broadcast rows
    # per-partition tables [128, ncols] f32
    tab_bq = din("tab_bq", [128, NL * 4], F32)
    tab_bk = din("tab_bk", [128, NL * 4], F32)
    tab_bo = din("tab_bo", [128, NL * 4], F32)
    tab_b2 = din("tab_b2", [128, NL * 4], F32)
    tab_b1 = din("tab_b1", [128, NL * 16], F32)
    tab_g1 = din("tab_g1", [128, NL * 4], F32)
    tab_o1 = din("tab_o1", [128, NL * 4], F32)
    tab_g2 = din("tab_g2", [128, NL * 4], F32)
    tab_o2 = din("tab_o2", [128, NL * 4], F32)
    tab_cn = din("tab_cn", [128, 8], F32)             # cn_g | cn_b (4+4)
    tab_cs = din("tab_cs", [128, 16], F32)            # downb|convb3|bng3|bnb3|upb4 + pad
    ones128_d = din("ones128_d", [128, 1])
    ones1_d = din("ones1_d", [1, 128])
    ones8_d = din("ones8_d", [128, 8])
    out_d = nc.dram_tensor("out", [LB, 4, 128, L], F32R, kind="ExternalOutput")

    with tile.TileContext(nc) as tc:
        _emit(nc, tc, live, locals())
    nc.compile()
    return nc


def _emit(nc, tc, live, D):
    v, sc, te, sy = nc.vector, nc.scalar, nc.tensor, nc.sync
    AO = AluOpType

    x25t, w25, cembT = D["x25t"], D["w25"], D["cembT"]
    downw, convk, upw = D["downw"], D["convk"], D["upw"]
    maskb, e8 = D["maskb"], D["e8"]
    wq, wk, wv, wo, w1, w2 = D["wq"], D["wk"], D["wv"], D["wo"], D["w1"], D["w2"]
    bvB, out_d = D["bvB"], D["out_d"]
    ones128_d, ones1_d, ones8_d = D["ones128_d"], D["ones1_d"], D["ones8_d"]

    import contextlib
    ctx = contextlib.ExitStack()
    with ctx:
        persist = ctx.enter_context(tc.tile_pool(name="persist", bufs=1))
        pp = ctx.enter_context(tc.tile_pool(name="ps", bufs=1, space="PSUM"))

        def psum(name):
            return pp.tile([128, 512], F32, tag="p1", bufs=8, name=name)

        # ---------- persistent constants ----------
        mb = []
        for r in range(6):
            t = persist.tile([128, L], F32, name=f"mb{r}")
            sy.dma_start(t[:], maskb.ap()[r])
            mb.append(t)
        e8_sb = persist.tile([8, DM], F32R, name="e8_sb")
        sy.dma_start(e8_sb[:], e8.ap())
        ones128 = persist.tile([128, 1], F32R, name="ones128")
        sy.dma_start(ones128[:], ones128_d.ap())
        ones1 = persist.tile([1, 128], F32R, name="ones1")
        sy.dma_start(ones1[:], ones1_d.ap())
        ones8 = persist.tile([128, 8], F32R, name="ones8")
        sy.dma_start(ones8[:], ones8_d.ap())
        epsT = persist.tile([1, 1], F32, name="epsT")
        v.memset(epsT[:], 1e-5)
        eps128 = persist.tile([128, 1], F32, name="eps128")
        v.memset(eps128[:], 1e-5)
        tabs = {}
        for nm in ("tab_bq", "tab_bk", "tab_bo", "tab_b2", "tab_b1", "tab_g1",
                   "tab_o1", "tab_g2", "tab_o2", "tab_cn", "tab_cs"):
            dr = D[nm]
            t = persist.tile(list(dr.shape), F32, name=nm + "_sb")
            sy.dma_start(t[:], dr.ap())
            tabs[nm] = t
        x_sb = [[persist.tile([128, L], F32R, name=f"x_{b}_{d}")
                 for d in range(4)] for b in range(LB)]

        # ================= Phase 1-3: embedding + CSCM =================
        with tc.tile_pool(name="cscm", bufs=1) as cp:
            w25_sb = cp.tile([25, DM], F32R, name="w25_sb")
            sy.dma_start(w25_sb[:], w25.ap())
            cemb_sb = [cp.tile([128, SEQ], F32, name=f"cemb{d}") for d in range(4)]
            for d in range(4):
                sy.dma_start(cemb_sb[d][:], cembT.ap()[d])
            downw_sb = [cp.tile([128, 128], F32R, name=f"dw{k}") for k in range(4)]
            for k in range(4):
                sy.dma_start(downw_sb[k][:], downw.ap()[k])
            convk_sb = [[cp.tile([128, 128], F32R, name=f"ck{i}_{w}")
                         for w in range(4)] for i in range(3)]
            for i in range(3):
                for w in range(4):
                    sy.dma_start(convk_sb[i][w][:], convk.ap()[i, w])
            upw_sb = cp.tile([128, DM], F32R, name="upw_sb")
            sy.dma_start(upw_sb[:], upw.ap())

            dall = cp.tile([128, NB * SEQ], F32R, name="dall")
            xcat = [[cp.tile([128, L], F32R, name=f"xc{b}_{d}")
                     for d in range(4)] for b in range(LB)]

            tab_cs = tabs["tab_cs"]
            downb = tab_cs[:, 0:1]

            # --- embedding + down-proj for all 16 batches ---
            for b in range(NB):
                x25_sb = cp.tile([25, SEQ], F32R, tag="x25", bufs=2, name=f"x25_{b}")
                sy.dma_start(x25_sb[:], x25t.ap()[b])
                etiles = []
                for d in range(4):
                    pe_ = psum(f"pe{b}_{d}")
                    te.matmul(pe_[:], w25_sb[:, d * 128:(d + 1) * 128], x25_sb[:],
                              start=True, stop=True)
                    if b < LB:
                        et = xcat[b][d][:, 0:SEQ]
                    else:
                        et_t = cp.tile([128, SEQ], F32R, tag="etile", bufs=8,
                                       name=f"et{b}_{d}")
                        et = et_t[:]
                    v.tensor_tensor(et, pe_[:], cemb_sb[d][:], AO.add)
                    etiles.append(et)
                pd_ = psum(f"pd{b}")
                for d in range(4):
                    te.matmul(pd_[:], downw_sb[d][:], etiles[d],
                              start=(d == 0), stop=(d == 3))
                sc.activation(dall[:, b * SEQ:(b + 1) * SEQ], pd_[:],
                              Act.Identity, bias=downb)

            # --- conv pyramid with training-mode BN (all batches) ---
            def bn_elu(pre, n_elem, lvl, out_name):
                # pre: f32 [128, n_elem]; returns f32r ELU(BN(pre)) tile
                s1 = cp.tile([128, 1], F32, tag="r1", bufs=4, name=f"s1_{lvl}")
                v.tensor_reduce(s1[:], pre, mybir.AxisListType.X, op=AO.add)
                mean = cp.tile([128, 1], F32, tag="r1", bufs=4, name=f"mn_{lvl}")
                sc.activation(mean[:], s1[:], Act.Copy, scale=1.0 / n_elem)
                cent = cp.tile([128, n_elem], F32, tag="bnc", bufs=1, name=f"ct_{lvl}")
                v.tensor_scalar(cent[:], pre, mean[:], None, AO.subtract)
                sq = cp.tile([128, n_elem], F32, tag="bnsq", bufs=1, name=f"sq_{lvl}")
                ssq = cp.tile([128, 1], F32, tag="r1", bufs=4, name=f"ssq_{lvl}")
                sc.activation(sq[:], cent[:], Act.Square, accum_out=ssq[:])
                var = cp.tile([128, 1], F32, tag="r1", bufs=4, name=f"vr_{lvl}")
                sc.activation(var[:], ssq[:], Act.Sqrt, bias=eps128[:],
                              scale=1.0 / n_elem)
                rstd = cp.tile([128, 1], F32, tag="r1", bufs=4, name=f"rs_{lvl}")
                v.reciprocal(rstd[:], var[:])
                seff = cp.tile([128, 1], F32, tag="r1", bufs=4, name=f"se_{lvl}")
                v.tensor_tensor(seff[:], rstd[:], tab_cs[:, 1 + lvl:2 + lvl], AO.mult)
                # z = cent*seff + bnb ; elu = relu(z) + exp(min(z,0)) - 1
                z = cp.tile([128, n_elem], F32, tag="bnc2", bufs=1, name=f"z_{lvl}")
                v.tensor_scalar(z[:], cent[:], seff[:], tab_cs[:, 7 + lvl:8 + lvl],
                                AO.mult, AO.add)
                zm = cp.tile([128, n_elem], F32, tag="bnsq", bufs=1, name=f"zm_{lvl}")
                v.tensor_scalar(zm[:], z[:], 0.0, None, AO.min)
                ze = cp.tile([128, n_elem], F32, tag="bnc", bufs=1, name=f"ze_{lvl}")
                sc.activation(ze[:], zm[:], Act.Exp)
                zr = cp.tile([128, n_elem], F32, tag="bnsq", bufs=1, name=f"zr_{lvl}")
                v.tensor_scalar(zr[:], z[:], 0.0, None, AO.max)
                eo = cp.tile([128, n_elem], F32R, name=out_name)
                v.scalar_tensor_tensor(eo[:], ze[:], -1.0, zr[:], AO.add, AO.add)
                return eo

            # level 1: input dall per batch [:, 512b + w::4] (128 cols)
            c1pre = cp.tile([128, NB * 128], F32, name="c1pre")
            for b in range(NB):
                pc = psum(f"pc1_{b}")
                for w in range(4):
                    rhs = dall[:, b * SEQ + w: b * SEQ + SEQ: 4]
                    te.matmul(pc[:, 0:128], convk_sb[0][w][:], rhs,
                              start=(w == 0), stop=(w == 3))
                sc.activation(c1pre[:, b * 128:(b + 1) * 128], pc[:, 0:128],
                              Act.Identity, bias=tab_cs[:, 4:5])
            c1e = bn_elu(c1pre[:], NB * 128, 0, "c1e")

            c2pre = cp.tile([128, NB * 32], F32, name="c2pre")
            for b in range(NB):
                pc = psum(f"pc2_{b}")
                for w in range(4):
                    rhs = c1e[:, b * 128 + w: b * 128 + 128: 4]
                    te.matmul(pc[:, 0:32], convk_sb[1][w][:], rhs,
                              start=(w == 0), stop=(w == 3))
                sc.activation(c2pre[:, b * 32:(b + 1) * 32], pc[:, 0:32],
                              Act.Identity, bias=tab_cs[:, 5:6])
            c2e = bn_elu(c2pre[:], NB * 32, 1, "c2e")

            c3pre = cp.tile([128, NB * 8], F32, name="c3pre")
            for b in range(NB):
                pc = psum(f"pc3_{b}")
                for w in range(4):
                    rhs = c2e[:, b * 32 + w: b * 32 + 32: 4]
                    te.matmul(pc[:, 0:8], convk_sb[2][w][:], rhs,
                              start=(w == 0), stop=(w == 3))
                sc.activation(c3pre[:, b * 8:(b + 1) * 8], pc[:, 0:8],
                              Act.Identity, bias=tab_cs[:, 6:7])
            c3e = bn_elu(c3pre[:], NB * 8, 2, "c3e")

            # --- up-proj + concat (local batches), then LN -> x_sb ---
            tab_cn = tabs["tab_cn"]
            for b in range(LB):
                cat = cp.tile([128, 168], F32R, tag="cat", bufs=2, name=f"cat{b}")
                v.tensor_copy(cat[:, 0:128], c1e[:, b * 128:(b + 1) * 128])
                v.tensor_copy(cat[:, 128:160], c2e[:, b * 32:(b + 1) * 32])
                v.tensor_copy(cat[:, 160:168], c3e[:, b * 8:(b + 1) * 8])
                for d in range(4):
                    pu = psum(f"pu{b}_{d}")
                    te.matmul(pu[:, 0:168], upw_sb[:, d * 128:(d + 1) * 128], cat[:],
                              start=True, stop=True)
                    sc.activation(xcat[b][d][:, SEQ:L], pu[:, 0:168],
                                  Act.Identity, bias=tab_cs[:, 12 + d:13 + d])
                # LN over d_model (partition dim) on xcat -> x_sb, two halves
                for hf in range(2):
                    _part_ln(nc, cp, psum, xcat[b], hf,
                             tab_cn[:, 0:4], tab_cn[:, 4:8],
                             [x_sb[b][d][:, hf * HW:(hf + 1) * HW] for d in range(4)],
                             ones128, ones1, epsT, f"cn{b}_{hf}")

        # ================= Phase 4: encoder layers =================
        with tc.tile_pool(name="wffn", bufs=1) as wf, \
             tc.tile_pool(name="act", bufs=1) as ap_:
            def wtiles(dram, l, n, width, tag):
                ts = []
                for k in range(n):
                    t = wf.tile([128, width], F32R, tag=tag, bufs=n,
                                name=f"{tag}{l}_{k}")
                    sy.dma_start(t[:], dram.ap()[l, k * 128:(k + 1) * 128, :])
                    ts.append(t)
                return ts

            for l in range(NL):
                wq_sb = wtiles(wq, l, 4, DM, "wq")
                wk_sb = wtiles(wk, l, 4, DM, "wk")
                wv_sb = wtiles(wv, l, 4, DM, "wv")
                wo_sb = wtiles(wo, l, 4, DM, "wo")
                w1_sb = []
                for k in range(4):
                    t = wf.tile([128, DFF], F32R, tag="w1", bufs=4, name=f"w1_{l}_{k}")
                    sy.dma_start(t[:], w1.ap()[l, k * 128:(k + 1) * 128, :])
                    w1_sb.append(t)
                w2_sb = []
                for m in range(16):
                    t = wf.tile([128, DM], F32R, tag="w2", bufs=16, name=f"w2_{l}_{m}")
                    sy.dma_start(t[:], w2.ap()[l, m * 128:(m + 1) * 128, :])
                    w2_sb.append(t)
                bv_sb = ap_.tile([128, DM], F32, tag="bv", bufs=1, name=f"bv{l}")
                sy.dma_start(bv_sb[:], bvB.ap()[l])

                for b in range(LB):
                    _layer(nc, tc, ap_, psum, live, l, b, x_sb[b],
                           wq_sb, wk_sb, wv_sb, wo_sb, w1_sb, w2_sb, bv_sb,
                           tabs, mb, e8_sb, ones128, ones1, epsT, ones8)

        # ================= Phase 5: dump x^T =================
        for b in range(LB):
            for d in range(4):
                sy.dma_start(out_d.ap()[b, d], x_sb[b][d][:])


def _part_ln(nc, pool, psum, src_tiles, hf, g_tab, b_tab, out_aps,
             ones128, ones1, epsT, nm):
    """LayerNorm over the partition (d_model) dim for one query-half.

    src_tiles: 4 tiles [128, >=680] f32r; uses cols [hf*HW, hf*HW+HW).
    out_aps: 4 destination APs [128, HW] (f32r).
    """
    v, sc, te = nc.vector, nc.scalar, nc.tensor
    AO = AluOpType
    hs = slice(hf * HW, (hf + 1) * HW)
    ps_s = psum(f"lns_{nm}")
    for d in range(4):
        te.matmul(ps_s[0:1, 0:HW], ones128[:], src_tiles[d][:, hs],
                  start=(d == 0), stop=(d == 3))
    mrow = pool.tile([1, HW], F32R, tag="lnrow", bufs=2, name=f"mr_{nm}")
    sc.activation(mrow[:], ps_s[0:1, 0:HW], Act.Copy, scale=1.0 / 512.0)
    ps_q = psum(f"lnq_{nm}")
    for d in range(4):
        sq = pool.tile([128, HW], F32R, tag="lnsq", bufs=2, name=f"sqt_{nm}_{d}")
        sc.activation(sq[:], src_tiles[d][:, hs], Act.Square)
        te.matmul(ps_q[0:1, 0:HW], ones128[:], sq[:],
                  start=(d == 0), stop=(d == 3))
    m2 = pool.tile([1, HW], F32, tag="lnrow2", bufs=2, name=f"m2_{nm}")
    v.tensor_tensor(m2[:], mrow[:], mrow[:], AO.mult)
    vrow = pool.tile([1, HW], F32, tag="lnrow2", bufs=2, name=f"vr_{nm}")
    v.scalar_tensor_tensor(vrow[:], ps_q[0:1, 0:HW], 1.0 / 512.0, m2[:],
                           AO.mult, AO.subtract)
    srow = pool.tile([1, HW], F32, tag="lnrow2", bufs=2, name=f"sr_{nm}")
    sc.activation(srow[:], vrow[:], Act.Sqrt, bias=epsT[:])
    rrow = pool.tile([1, HW], F32, tag="lnrow2", bufs=2, name=f"rr_{nm}")
    v.reciprocal(rrow[:], srow[:])
    rrow_r = pool.tile([1, HW], F32R, tag="lnrow", bufs=2, name=f"rrr_{nm}")
    v.tensor_copy(rrow_r[:], rrow[:])
    ps_m = psum(f"lnm_{nm}")
    te.matmul(ps_m[:, 0:HW], ones1[:], mrow[:], start=True, stop=True)
    ps_r = psum(f"lnr_{nm}")
    te.matmul(ps_r[:, 0:HW], ones1[:], rrow_r[:], start=True, stop=True)
    for d in range(4):
        t1 = pool.tile([128, HW], F32, tag="lnt1", bufs=2, name=f"t1_{nm}_{d}")
        v.scalar_tensor_tensor(t1[:], src_tiles[d][:, hs], 1.0, ps_m[:, 0:HW],
                               AO.bypass, AO.subtract)
        t2 = pool.tile([128, HW], F32, tag="lnt2", bufs=2, name=f"t2_{nm}_{d}")
        v.tensor_tensor(t2[:], t1[:], ps_r[:, 0:HW], AO.mult)
        v.tensor_scalar(out_aps[d], t2[:], g_tab[:, d:d + 1], b_tab[:, d:d + 1],
                        AO.mult, AO.add)


def _layer(nc, tc, ap_, psum, live, l, b, xb,
           wq_sb, wk_sb, wv_sb, wo_sb, w1_sb, w2_sb, bv_sb,
           tabs, mb, e8_sb, ones128, ones1, epsT, ones8=None):
    v, sc, te, sy = nc.vector, nc.scalar, nc.tensor, nc.sync
    AO = AluOpType
    t_bq, t_bk = tabs["tab_bq"], tabs["tab_bk"]
    t_bo, t_b2, t_b1 = tabs["tab_bo"], tabs["tab_b2"], tabs["tab_b1"]
    t_g1, t_o1 = tabs["tab_g1"], tabs["tab_o1"]
    t_g2, t_o2 = tabs["tab_g2"], tabs["tab_o2"]

    # ---- K^T (full width) and V+ (token-major) ----
    kT = [ap_.tile([128, L], F32R, tag="kT", bufs=4, name=f"kT{l}{b}_{d}")
          for d in range(4)]
    for hf in range(2):
        hs = slice(hf * HW, (hf + 1) * HW)
        for d in range(4):
            pk = psum(f"pk{l}{b}{hf}{d}")
            for k in range(4):
                te.matmul(pk[:, 0:HW], wk_sb[k][:, d * 128:(d + 1) * 128],
                          xb[k][:, hs], start=(k == 0), stop=(k == 3))
            sc.activation(kT[d][:, hs], pk[:, 0:HW], Act.Identity,
                          bias=t_bk[:, l * 4 + d:l * 4 + d + 1])
    vplus = []
    for t in range(6):
        a, bb = KT[t]
        w = bb - a
        vt = ap_.tile([128, 520], F32R, tag="vplus", bufs=6, name=f"vp{l}{b}_{t}")
        pv = psum(f"pv{l}{b}{t}")
        for k in range(4):
            te.matmul(pv[:w, 0:DM], xb[k][:, a:bb], wv_sb[k][:],
                      start=(k == 0), stop=(k == 3))
        ov = vt[:w].rearrange("p (h j) -> p h j", h=8)[:, :, 0:64]
        pvv = pv[:w, 0:DM].rearrange("p (h j) -> p h j", h=8)
        bvv = bv_sb[:w].rearrange("p (h j) -> p h j", h=8)
        v.tensor_tensor(ov, pvv, bvv, AO.add)
        v.tensor_copy(vt[:w].rearrange("p (h j) -> p h j", h=8)[:, :, 64:65],
                      ones8[:w].unsqueeze(2))
        vplus.append(vt)

    for hf in range(2):
        hs = slice(hf * HW, (hf + 1) * HW)
        # ---- Q^T for this half ----
        qTh = [ap_.tile([128, HW], F32R, tag="qTh", bufs=4, name=f"qT{l}{b}{hf}_{d}")
               for d in range(4)]
        for d in range(4):
            pq = psum(f"pq{l}{b}{hf}{d}")
            for k in range(4):
                te.matmul(pq[:, 0:HW], wq_sb[k][:, d * 128:(d + 1) * 128],
                          xb[k][:, hs], start=(k == 0), stop=(k == 3))
            sc.activation(qTh[d][:, :], pq[:, 0:HW], Act.Identity,
                          bias=t_bq[:, l * 4 + d:l * 4 + d + 1])

        rs = [r for r in range(6) if live[(r, hf)]]
        oTh = [ap_.tile([128, HW], F32R, tag="oTh", bufs=4, name=f"oT{l}{b}{hf}_{d}")
               for d in range(4)]
        dnm = ap_.tile([8, HW], F32, tag="dnm", bufs=2, name=f"dn{l}{b}{hf}")
        for h in range(NH):
            d4, r64 = h // 2, (h % 2) * 64
            po = psum(f"po{l}{b}{hf}{h}")
            for ri, r in enumerate(rs):
                a, bb = KT[r]
                kp = bb - a
                ps_ = psum(f"ps{l}{b}{hf}{h}{r}")
                te.matmul(ps_[:kp, 0:HW], kT[d4][r64:r64 + 64, a:bb],
                          qTh[d4][r64:r64 + 64, :], start=True, stop=True)
                st = ap_.tile([128, HW], F32, tag="stmp", bufs=2,
                              name=f"st{l}{b}{hf}{h}{r}")
                v.tensor_tensor(st[:kp], ps_[:kp, 0:HW], mb[r][:kp, hs], AO.add)
                se = ap_.tile([128, HW], F32R, tag="sexp", bufs=2,
                              name=f"se{l}{b}{hf}{h}{r}")
                sc.activation(se[:kp], st[:kp], Act.Exp)
                te.matmul(po[0:65, 0:HW], vplus[r][:kp, h * 65:h * 65 + 65],
                          se[:kp], start=(ri == 0), stop=(ri == len(rs) - 1))
            sc.activation(oTh[d4][r64:r64 + 64, :], po[0:64, 0:HW], Act.Copy)
            dstage = ap_.tile([1, HW], F32, tag="dstage", bufs=2,
                              name=f"dg{l}{b}{hf}{h}")
            sc.activation(dstage[:], po[64:65, 0:HW], Act.Copy)
            sy.dma_start(dnm[h:h + 1, :], dstage[:])
        # ---- normalize O by softmax denominators ----
        dnr = ap_.tile([8, HW], F32, tag="dnm", bufs=2, name=f"dr{l}{b}{hf}")
        v.reciprocal(dnr[:], dnm[:])
        dnr_r = ap_.tile([8, HW], F32R, tag="dnmr", bufs=1, name=f"drr{l}{b}{hf}")
        v.tensor_copy(dnr_r[:], dnr[:])
        for m in range(4):
            prb = psum(f"prb{l}{b}{hf}{m}")
            te.matmul(prb[:, 0:HW], e8_sb[:, m * 128:(m + 1) * 128], dnr_r[:],
                      start=True, stop=True)
            v.tensor_tensor(oTh[m][:], oTh[m][:], prb[:, 0:HW], AO.mult)
        # ---- Wo proj + residual -> LN1 -> x ----
        resid = [ap_.tile([128, HW], F32R, tag="resid", bufs=4,
                          name=f"rs{l}{b}{hf}_{d}") for d in range(4)]
        for d in range(4):
            pa = psum(f"pa{l}{b}{hf}{d}")
            for k in range(4):
                te.matmul(pa[:, 0:HW], wo_sb[k][:, d * 128:(d + 1) * 128],
                          oTh[k][:], start=(k == 0), stop=(k == 3))
            v.scalar_tensor_tensor(resid[d][:], pa[:, 0:HW],
                                   t_bo[:, l * 4 + d:l * 4 + d + 1],
                                   xb[d][:, hs], AO.add, AO.add)
        _part_ln(nc, ap_, psum, resid, 0,
                 t_g1[:, l * 4:l * 4 + 4], t_o1[:, l * 4:l * 4 + 4],
                 [xb[d][:, hs] for d in range(4)],
                 ones128, ones1, epsT, f"l1_{l}{b}{hf}")
        # ---- FFN ----
        py = [psum(f"py{l}{b}{hf}{d}") for d in range(4)]
        for m in range(16):
            ph = psum(f"ph{l}{b}{hf}{m}")
            for k in range(4):
                te.matmul(ph[:, 0:HW], w1_sb[k][:, m * 128:(m + 1) * 128],
                          xb[k][:, hs], start=(k == 0), stop=(k == 3))
            hT = ap_.tile([128, HW], F32R, tag="hT", bufs=2, name=f"h{l}{b}{hf}{m}")
            sc.activation(hT[:], ph[:, 0:HW], Act.Gelu,
                          bias=t_b1[:, l * 16 + m:l * 16 + m + 1])
            for d in range(4):
                te.matmul(py[d][:, 0:HW], w2_sb[m][:, d * 128:(d + 1) * 128],
                          hT[:], start=(m == 0), stop=(m == 15))
        resid2 = [ap_.tile([128, HW], F32R, tag="resid", bufs=4,
                           name=f"r2{l}{b}{hf}_{d}") for d in range(4)]
        for d in range(4):
            v.scalar_tensor_tensor(resid2[d][:], py[d][:, 0:HW],
                                   t_b2[:, l * 4 + d:l * 4 + d + 1],
                                   xb[d][:, hs], AO.add, AO.add)
        _part_ln(nc, ap_, psum, resid2, 0,
                 t_g2[:, l * 4:l * 4 + 4], t_o2[:, l * 4:l * 4 + 4],
                 [xb[d][:, hs] for d in range(4)],
                 ones128, ones1, epsT, f"l2_{l}{b}{hf}")


# ======================= host side =======================
_PROG = None


def _pos_embed(n, d):
    pos = np.arange(n, dtype=np.float32)[:, None]
    div = np.exp(np.arange(0, d, 2, dtype=np.float32) * (-np.log(10000.0) / d))
    pe = np.zeros((n, d), dtype=np.float32)
    pe[:, 0::2] = np.sin(pos * div)
    pe[:, 1::2] = np.cos(pos * div)
    return pe


def _padtab(a, rows=128):
    # a: [n, cols] -> [128, cols] zero-padded
    out = np.zeros((rows, a.shape[1]), np.float32)
    out[:a.shape[0]] = a
    return out


def kernel(**inputs):
    global _PROG
    inputs = {k: np.asarray(v) for k, v in inputs.items()}
    attn_mask = inputs["attn_mask"]
    live = _nonempty(attn_mask)
    if _PROG is None:
        _PROG = _build_program(live)
    nc = _PROG

    x_enc = inputs["x_enc"].astype(np.float32)
    x_mark = inputs["x_mark_enc"].astype(np.float32)
    tok = inputs["tok_kernel"].astype(np.float32)

    X25 = np.concatenate([np.roll(x_enc, 1, axis=1), x_enc,
                          np.roll(x_enc, -1, axis=1), x_mark], axis=2)  # [B,512,25]
    X25T = np.ascontiguousarray(X25.transpose(0, 2, 1))                 # [B,25,512]
    W25 = np.concatenate([tok[0], tok[1], tok[2], inputs["mark_W"]], axis=0)
    Cemb = _pos_embed(SEQ, DM) + inputs["mark_b"]                       # [512,512]
    CembT = np.ascontiguousarray(Cemb.T).reshape(4, 128, SEQ)

    maskb = np.zeros((6, 128, L), np.float32)
    biasT = _mask_bias(attn_mask)  # [680, 680] (k, q) == transpose (symmetric)
    for r, (a, bb) in enumerate(KT):
        maskb[r, :bb - a] = biasT[a:bb]

    e8 = np.zeros((8, DM), np.float32)
    for h in range(8):
        e8[h, h * 64:(h + 1) * 64] = 1.0

    com = dict(
        w25=W25, cembT=CembT,
        downw=np.ascontiguousarray(inputs["down_W"].reshape(4, 128, 128)),
        convk=inputs["conv_K"].astype(np.float32),
        upw=inputs["up_W"].astype(np.float32),
        maskb=maskb, e8=e8,
        wq=inputs["Wq"] / 8.0, wk=inputs["Wk"], wv=inputs["Wv"], wo=inputs["Wo"],
        w1=inputs["W1"], w2=inputs["W2"],
        bvB=np.broadcast_to(inputs["bv"][:, None, :], (NL, 128, DM)).copy(),
        tab_bq=(inputs["bq"] / 8.0).reshape(NL * 4, 128).T.copy(),
        tab_bk=inputs["bk"].reshape(NL * 4, 128).T.copy(),
        tab_bo=inputs["bo"].reshape(NL * 4, 128).T.copy(),
        tab_b2=inputs["b2"].reshape(NL * 4, 128).T.copy(),
        tab_b1=inputs["b1"].reshape(NL * 16, 128).T.copy(),
        tab_g1=inputs["ln1_g"].reshape(NL * 4, 128).T.copy(),
        tab_o1=inputs["ln1_b"].reshape(NL * 4, 128).T.copy(),
        tab_g2=inputs["ln2_g"].reshape(NL * 4, 128).T.copy(),
        tab_o2=inputs["ln2_b"].reshape(NL * 4, 128).T.copy(),
        ones128_d=np.ones((128, 1), np.float32),
        ones1_d=np.ones((1, 128), np.float32),
        ones8_d=np.ones((128, 8), np.float32),
        tab_cn=np.concatenate([inputs["cn_g"].reshape(4, 128).T,
                               inputs["cn_b"].reshape(4, 128).T], axis=1).copy(),
    )
    cs = np.zeros((128, 16), np.float32)
    cs[:, 0] = inputs["down_b"]
    for i in range(3):
        cs[:, 1 + i] = inputs["bn_g"][i]
        cs[:, 4 + i] = inputs["conv_b"][i]
        cs[:, 7 + i] = inputs["bn_b"][i]
    cs[:, 12:16] = inputs["up_b"].reshape(4, 128).T
    com["tab_cs"] = cs
    com = {k: np.ascontiguousarray(v, np.float32) for k, v in com.items()}

    in_maps = []
    for c in range(N_CORES):
        order = [2 * c, 2 * c + 1] + [i for i in range(NB) if i not in (2 * c, 2 * c + 1)]
        m = dict(com)
        m["x25t"] = np.ascontiguousarray(X25T[order])
        in_maps.append(m)

    res = run_bass_kernel_spmd(nc, in_maps, core_ids=list(range(N_CORES)))

    # assemble: out per core [2, 4, 128, 680] feature-major -> [B, 680, 512]
    X = np.empty((NB, L, DM), np.float32)
    for c in range(N_CORES):
        o = res.results[c]["out"]  # [2, 4, 128, 680]
        for j in range(LB):
            X[2 * c + j] = o[j].reshape(DM, L).T
    gidx = np.asarray(inputs["gather_idx"]).astype(np.int64)
    out = X[:, gidx, :].reshape(NB, SEQ, NH * 4 * DKH)
    return out.astype(np.float32)


# revision 13
# speedup vs baseline: 1.0674x; 1.0674x over previous
"""Bass/Trainium2 kernel for nn_Encoder_47210280517649 (Pyraformer encoder).

Strategy: pure data-parallel over batch (B=16 -> 2 per core across 8 cores).
Feature-major activations x^T [d_model(4x128 part), 680 tok(free)], fp32r
matmuls, CSCM/BatchNorm computed redundantly over all 16 batches on every
core (keeps BN exact without collectives), attention computed as S^T per
key-tile with softmax denominators obtained from a ones-column appended to V
and normalization broadcast via an indicator matmul.
"""
import sys
import numpy as np

try:
    import concourse  # noqa: F401
except ImportError:
    for _p in ("/opt/trn_rl_repo", "/root/.axon_site/_ro/trn_rl_repo"):
        sys.path.insert(0, _p)

import concourse.bass as bass  # noqa: E402
import concourse.mybir as mybir  # noqa: E402
import concourse.tile as tile  # noqa: E402
from concourse import bacc  # noqa: E402
from concourse.bass_utils import run_bass_kernel_spmd  # noqa: E402
from concourse.alu_op_type import AluOpType  # noqa: E402

dt = mybir.dt
F32, F32R = dt.float32, dt.float32r
Act = mybir.ActivationFunctionType

N_CORES = 8
NB = 16          # total batch
LB = 2           # local batch per core
SEQ = 512
DM = 512         # d_model
L = 680          # 512+128+32+8
NH = 8
DKH = 64
DFF = 2048
NL = 4
HW = 340         # query-half width
KT = [(0, 128), (128, 256), (256, 384), (384, 512), (512, 640), (640, 680)]
NEG = -10000.0

# token tiles for V (token-major)
VT = KT


def _mask_bias(attn_mask: np.ndarray) -> np.ndarray:
    return np.where(attn_mask, np.float32(NEG), np.float32(0.0))


def _windows(attn_mask):
    """Per (k-tile r, q-half hf): merged column ranges with any kept entry.

    Returns (win, totw): win[(r, hf)] = [(q0_rel, w, packoff), ...];
    packoff indexes into the packed mask-bias array of total width totw.
    """
    keep = ~attn_mask
    win = {}
    off = 0
    for r, (a, b) in enumerate(KT):
        for hf in range(2):
            cols = keep[a:b, hf * HW:(hf + 1) * HW].any(axis=0)
            rs = []
            i = 0
            while i < HW:
                if cols[i]:
                    j = i
                    while j < HW and (cols[j] or (j + 32 < HW and cols[j:j + 32].any())):
                        j += 1
                    rs.append((i, j - i))
                    i = j
                else:
                    i += 1
            lst = []
            for (q0, w) in rs:
                lst.append((q0, w, off))
                off += w
            win[(r, hf)] = lst
    return win, off


def _nonempty(attn_mask):
    """(r, half) pairs that have at least one kept (non-blocked) entry."""
    keep = ~attn_mask
    live = {}
    for r, (a, b) in enumerate(KT):
        for h in range(2):
            live[(r, h)] = bool(keep[a:b, h * HW:(h + 1) * HW].any())
    return live


def _build_program(live, win, totw):
    nc = bacc.Bacc("TRN2", target_bir_lowering=False, debug=False,
                   num_devices=N_CORES)

    def din(name, shape, dtp=F32R):
        return nc.dram_tensor(name, shape, dtp, kind="ExternalInput")

    # ---- DRAM inputs ----
    x25t = din("x25t", [NB, 25, SEQ])                 # folded conv+mark input
    w25 = din("w25", [25, DM])
    cembT = din("cembT", [4, 128, SEQ], F32)          # pos+mark_b, feat-major
    downw = din("downw", [4, 128, 128])
    convk = din("convk", [3, 4, 128, 128])
    upw = din("upw", [128, DM])
    maskw = din("maskw", [128, max(totw, 1)], F32)    # packed windowed mask bias
    zeros_d = din("zeros_d", [128, HW])
    e8 = din("e8", [8, DM])                           # head indicator
    # weights per layer
    wq = din("wq", [NL, DM, DM]); wk = din("wk", [NL, DM, DM])
    wv = din("wv", [NL, DM, DM]); wo = din("wo", [NL, DM, DM])
    w1 = din("w1", [NL, DM, DFF]); w2 = din("w2", [NL, DFF, DM])
    bvB = din("bvB", [NL, 128, DM], F32)              # V bias broadcast rows
    # per-partition tables [128, ncols] f32
    tab_bq = din("tab_bq", [128, NL * 4], F32)
    tab_bk = din("tab_bk", [128, NL * 4], F32)
    tab_bo = din("tab_bo", [128, NL * 4], F32)
    tab_b2 = din("tab_b2", [128, NL * 4], F32)
    tab_b1 = din("tab_b1", [128, NL * 16], F32)
    tab_g1 = din("tab_g1", [128, NL * 4], F32)
    tab_o1 = din("tab_o1", [128, NL * 4], F32)
    tab_g2 = din("tab_g2", [128, NL * 4], F32)
    tab_o2 = din("tab_o2", [128, NL * 4], F32)
    tab_cn = din("tab_cn", [128, 8], F32)             # cn_g | cn_b (4+4)
    tab_cs = din("tab_cs", [128, 16], F32)            # downb|convb3|bng3|bnb3|upb4 + pad
    ones128_d = din("ones128_d", [128, 1])
    ones1_d = din("ones1_d", [1, 128])
    ones8_d = din("ones8_d", [128, 8])
    out_d = nc.dram_tensor("out", [LB, 4, 128, L], F32R, kind="ExternalOutput")

    with tile.TileContext(nc) as tc:
        _emit(nc, tc, live, win, locals())
    nc.compile()
    return nc


def _emit(nc, tc, live, win, D):
    v, sc, te, sy = nc.vector, nc.scalar, nc.tensor, nc.sync
    AO = AluOpType

    x25t, w25, cembT = D["x25t"], D["w25"], D["cembT"]
    downw, convk, upw = D["downw"], D["convk"], D["upw"]
    maskw, zeros_d, e8 = D["maskw"], D["zeros_d"], D["e8"]
    wq, wk, wv, wo, w1, w2 = D["wq"], D["wk"], D["wv"], D["wo"], D["w1"], D["w2"]
    bvB, out_d = D["bvB"], D["out_d"]
    ones128_d, ones1_d, ones8_d = D["ones128_d"], D["ones1_d"], D["ones8_d"]

    import contextlib
    ctx = contextlib.ExitStack()
    with ctx:
        persist = ctx.enter_context(tc.tile_pool(name="persist", bufs=1))
        pp = ctx.enter_context(tc.tile_pool(name="ps", bufs=1, space="PSUM"))

        def psum(name):
            return pp.tile([128, 512], F32, tag="p1", bufs=8, name=name)

        # ---------- persistent constants ----------
        mw_sb = persist.tile([128, maskw.shape[1]], F32, name="mw_sb")
        sy.dma_start(mw_sb[:], maskw.ap())
        e8_sb = persist.tile([8, DM], F32R, name="e8_sb")
        sy.dma_start(e8_sb[:], e8.ap())
        ones128 = persist.tile([128, 1], F32R, name="ones128")
        sy.dma_start(ones128[:], ones128_d.ap())
        ones1 = persist.tile([1, 128], F32R, name="ones1")
        sy.dma_start(ones1[:], ones1_d.ap())
        ones8 = persist.tile([128, 8], F32R, name="ones8")
        sy.dma_start(ones8[:], ones8_d.ap())
        epsT = persist.tile([1, 1], F32, name="epsT")
        v.memset(epsT[:], 1e-5)
        eps128 = persist.tile([128, 1], F32, name="eps128")
        v.memset(eps128[:], 1e-5)
        tabs = {}
        for nm in ("tab_bq", "tab_bk", "tab_bo", "tab_b2", "tab_b1", "tab_g1",
                   "tab_o1", "tab_g2", "tab_o2", "tab_cn", "tab_cs"):
            dr = D[nm]
            t = persist.tile(list(dr.shape), F32, name=nm + "_sb")
            sy.dma_start(t[:], dr.ap())
            tabs[nm] = t
        x_sb = [[persist.tile([128, L], F32R, name=f"x_{b}_{d}")
                 for d in range(4)] for b in range(LB)]

        # ================= Phase 1-3: embedding + CSCM =================
        with tc.tile_pool(name="cscm", bufs=1) as cp:
            w25_sb = cp.tile([25, DM], F32R, name="w25_sb")
            sy.dma_start(w25_sb[:], w25.ap())
            cemb_sb = [cp.tile([128, SEQ], F32, name=f"cemb{d}") for d in range(4)]
            for d in range(4):
                sy.dma_start(cemb_sb[d][:], cembT.ap()[d])
            downw_sb = [cp.tile([128, 128], F32R, name=f"dw{k}") for k in range(4)]
            for k in range(4):
                sy.dma_start(downw_sb[k][:], downw.ap()[k])
            convk_sb = [[cp.tile([128, 128], F32R, name=f"ck{i}_{w}")
                         for w in range(4)] for i in range(3)]
            for i in range(3):
                for w in range(4):
                    sy.dma_start(convk_sb[i][w][:], convk.ap()[i, w])
            upw_sb = cp.tile([128, DM], F32R, name="upw_sb")
            sy.dma_start(upw_sb[:], upw.ap())

            dall = cp.tile([128, NB * SEQ], F32R, name="dall")
            xcat = [[cp.tile([128, L], F32R, name=f"xc{b}_{d}")
                     for d in range(4)] for b in range(LB)]

            tab_cs = tabs["tab_cs"]
            downb = tab_cs[:, 0:1]

            # --- embedding + down-proj for all 16 batches ---
            for b in range(NB):
                x25_sb = cp.tile([25, SEQ], F32R, tag="x25", bufs=2, name=f"x25_{b}")
                sy.dma_start(x25_sb[:], x25t.ap()[b])
                etiles = []
                for d in range(4):
                    pe_ = psum(f"pe{b}_{d}")
                    te.matmul(pe_[:], w25_sb[:, d * 128:(d + 1) * 128], x25_sb[:],
                              start=True, stop=True)
                    if b < LB:
                        et = xcat[b][d][:, 0:SEQ]
                    else:
                        et_t = cp.tile([128, SEQ], F32R, tag="etile", bufs=8,
                                       name=f"et{b}_{d}")
                        et = et_t[:]
                    v.tensor_tensor(et, pe_[:], cemb_sb[d][:], AO.add)
                    etiles.append(et)
                pd_ = psum(f"pd{b}")
                for d in range(4):
                    te.matmul(pd_[:], downw_sb[d][:], etiles[d],
                              start=(d == 0), stop=(d == 3))
                sc.activation(dall[:, b * SEQ:(b + 1) * SEQ], pd_[:],
                              Act.Identity, bias=downb)

            # --- conv pyramid with training-mode BN (all batches) ---
            def bn_elu(pre, n_elem, lvl, out_name):
                # pre: f32 [128, n_elem]; returns f32r ELU(BN(pre)) tile
                s1 = cp.tile([128, 1], F32, tag="r1", bufs=4, name=f"s1_{lvl}")
                v.tensor_reduce(s1[:], pre, mybir.AxisListType.X, op=AO.add)
                mean = cp.tile([128, 1], F32, tag="r1", bufs=4, name=f"mn_{lvl}")
                sc.activation(mean[:], s1[:], Act.Copy, scale=1.0 / n_elem)
                cent = cp.tile([128, n_elem], F32, tag="bnc", bufs=1, name=f"ct_{lvl}")
                nc.gpsimd.tensor_scalar(cent[:], pre, mean[:], None, AO.subtract)
                sq = cp.tile([128, n_elem], F32, tag="bnsq", bufs=1, name=f"sq_{lvl}")
                ssq = cp.tile([128, 1], F32, tag="r1", bufs=4, name=f"ssq_{lvl}")
                sc.activation(sq[:], cent[:], Act.Square, accum_out=ssq[:])
                var = cp.tile([128, 1], F32, tag="r1", bufs=4, name=f"vr_{lvl}")
                sc.activation(var[:], ssq[:], Act.Sqrt, bias=eps128[:],
                              scale=1.0 / n_elem)
                rstd = cp.tile([128, 1], F32, tag="r1", bufs=4, name=f"rs_{lvl}")
                v.reciprocal(rstd[:], var[:])
                seff = cp.tile([128, 1], F32, tag="r1", bufs=4, name=f"se_{lvl}")
                v.tensor_tensor(seff[:], rstd[:], tab_cs[:, 1 + lvl:2 + lvl], AO.mult)
                # z = cent*seff + bnb ; elu = relu(z) + exp(min(z,0)) - 1
                z = cp.tile([128, n_elem], F32, tag="bnc2", bufs=1, name=f"z_{lvl}")
                v.tensor_scalar(z[:], cent[:], seff[:], tab_cs[:, 7 + lvl:8 + lvl],
                                AO.mult, AO.add)
                zm = cp.tile([128, n_elem], F32, tag="bnsq", bufs=1, name=f"zm_{lvl}")
                nc.gpsimd.tensor_scalar(zm[:], z[:], 0.0, None, AO.min)
                ze = cp.tile([128, n_elem], F32, tag="bnc", bufs=1, name=f"ze_{lvl}")
                sc.activation(ze[:], zm[:], Act.Exp)
                zr = cp.tile([128, n_elem], F32, tag="bnc3", bufs=1, name=f"zr_{lvl}")
                nc.gpsimd.tensor_scalar(zr[:], z[:], 0.0, None, AO.max)
                eo = cp.tile([128, n_elem], F32R, name=out_name)
                v.scalar_tensor_tensor(eo[:], ze[:], -1.0, zr[:], AO.add, AO.add)
                return eo

            # level 1: input dall per batch [:, 512b + w::4] (128 cols)
            c1pre = cp.tile([128, NB * 128], F32, name="c1pre")
            for b in range(NB):
                pc = psum(f"pc1_{b}")
                for w in range(4):
                    rhs = dall[:, b * SEQ + w: b * SEQ + SEQ: 4]
                    te.matmul(pc[:, 0:128], convk_sb[0][w][:], rhs,
                              start=(w == 0), stop=(w == 3))
                sc.activation(c1pre[:, b * 128:(b + 1) * 128], pc[:, 0:128],
                              Act.Identity, bias=tab_cs[:, 4:5])
            c1e = bn_elu(c1pre[:], NB * 128, 0, "c1e")

            c2pre = cp.tile([128, NB * 32], F32, name="c2pre")
            for b in range(NB):
                pc = psum(f"pc2_{b}")
                for w in range(4):
                    rhs = c1e[:, b * 128 + w: b * 128 + 128: 4]
                    te.matmul(pc[:, 0:32], convk_sb[1][w][:], rhs,
                              start=(w == 0), stop=(w == 3))
                sc.activation(c2pre[:, b * 32:(b + 1) * 32], pc[:, 0:32],
                              Act.Identity, bias=tab_cs[:, 5:6])
            c2e = bn_elu(c2pre[:], NB * 32, 1, "c2e")

            c3pre = cp.tile([128, NB * 8], F32, name="c3pre")
            for b in range(NB):
                pc = psum(f"pc3_{b}")
                for w in range(4):
                    rhs = c2e[:, b * 32 + w: b * 32 + 32: 4]
                    te.matmul(pc[:, 0:8], convk_sb[2][w][:], rhs,
                              start=(w == 0), stop=(w == 3))
                sc.activation(c3pre[:, b * 8:(b + 1) * 8], pc[:, 0:8],
                              Act.Identity, bias=tab_cs[:, 6:7])
            c3e = bn_elu(c3pre[:], NB * 8, 2, "c3e")

            # --- up-proj + concat (local batches), then LN -> x_sb ---
            tab_cn = tabs["tab_cn"]
            for b in range(LB):
                cat = cp.tile([128, 168], F32R, tag="cat", bufs=2, name=f"cat{b}")
                nc.gpsimd.tensor_copy(cat[:, 0:128], c1e[:, b * 128:(b + 1) * 128])
                nc.gpsimd.tensor_copy(cat[:, 128:160], c2e[:, b * 32:(b + 1) * 32])
                nc.gpsimd.tensor_copy(cat[:, 160:168], c3e[:, b * 8:(b + 1) * 8])
                for d in range(4):
                    pu = psum(f"pu{b}_{d}")
                    te.matmul(pu[:, 0:168], upw_sb[:, d * 128:(d + 1) * 128], cat[:],
                              start=True, stop=True)
                    sc.activation(xcat[b][d][:, SEQ:L], pu[:, 0:168],
                                  Act.Identity, bias=tab_cs[:, 12 + d:13 + d])
                # LN over d_model (partition dim) on xcat -> x_sb, two halves
                for hf in range(2):
                    _part_ln(nc, cp, psum, xcat[b], hf,
                             tab_cn[:, 0:4], tab_cn[:, 4:8],
                             [x_sb[b][d][:, hf * HW:(hf + 1) * HW] for d in range(4)],
                             ones128, ones1, epsT, f"cn{b}_{hf}")

        # ================= Phase 4: encoder layers =================
        with tc.tile_pool(name="wffn", bufs=1) as wf, \
             tc.tile_pool(name="act", bufs=1) as ap_:
            def wtiles(dram, l, n, width, tag):
                ts = []
                for k in range(n):
                    t = wf.tile([128, width], F32R, tag=tag, bufs=n,
                                name=f"{tag}{l}_{k}")
                    sy.dma_start(t[:], dram.ap()[l, k * 128:(k + 1) * 128, :])
                    ts.append(t)
                return ts

            sxp = {}
            for (r, hf), lst in win.items():
                if live[(r, hf)]:
                    t = ap_.tile([128, HW], F32R, name=f"sxp{r}_{hf}")
                    sy.dma_start(t[:], zeros_d.ap())
                    sxp[(r, hf)] = t

            for l in range(NL):
                wq_sb = wtiles(wq, l, 4, DM, "wq")
                wk_sb = wtiles(wk, l, 4, DM, "wk")
                wv_sb = wtiles(wv, l, 4, DM, "wv")
                wo_sb = wtiles(wo, l, 4, DM, "wo")
                w1_sb = []
                for k in range(4):
                    t = wf.tile([128, DFF], F32R, tag="w1", bufs=4, name=f"w1_{l}_{k}")
                    sy.dma_start(t[:], w1.ap()[l, k * 128:(k + 1) * 128, :])
                    w1_sb.append(t)
                w2_sb = []
                for m in range(16):
                    t = wf.tile([128, DM], F32R, tag="w2", bufs=16, name=f"w2_{l}_{m}")
                    sy.dma_start(t[:], w2.ap()[l, m * 128:(m + 1) * 128, :])
                    w2_sb.append(t)
                bv_sb = ap_.tile([128, DM], F32, tag="bv", bufs=1, name=f"bv{l}")
                sy.dma_start(bv_sb[:], bvB.ap()[l])

                for b in range(LB):
                    _layer(nc, tc, ap_, psum, live, l, b, x_sb[b],
                           wq_sb, wk_sb, wv_sb, wo_sb, w1_sb, w2_sb, bv_sb,
                           tabs, mw_sb, sxp, win, e8_sb, ones128, ones1,
                           epsT, ones8)

        # ================= Phase 5: dump x^T =================
        for b in range(LB):
            for d in range(4):
                sy.dma_start(out_d.ap()[b, d], x_sb[b][d][:])


def _part_ln(nc, pool, psum, src_tiles, hf, g_tab, b_tab, out_aps,
             ones128, ones1, epsT, nm):
    """LayerNorm over the partition (d_model) dim for one query-half.

    src_tiles: 4 tiles [128, >=680] f32r; uses cols [hf*HW, hf*HW+HW).
    out_aps: 4 destination APs [128, HW] (f32r).
    """
    v, sc, te = nc.vector, nc.scalar, nc.tensor
    AO = AluOpType
    hs = slice(hf * HW, (hf + 1) * HW)
    ps_s = psum(f"lns_{nm}")
    for d in range(4):
        te.matmul(ps_s[0:1, 0:HW], ones128[:], src_tiles[d][:, hs],
                  start=(d == 0), stop=(d == 3))
    mrow = pool.tile([1, HW], F32R, tag="lnrow", bufs=2, name=f"mr_{nm}")
    v.tensor_scalar(mrow[:], ps_s[0:1, 0:HW], 1.0 / 512.0, None, AO.mult)
    ps_q = psum(f"lnq_{nm}")
    for d in range(4):
        sq = pool.tile([128, HW], F32R, tag="lnsq", bufs=2, name=f"sqt_{nm}_{d}")
        sc.activation(sq[:], src_tiles[d][:, hs], Act.Square)
        te.matmul(ps_q[0:1, 0:HW], ones128[:], sq[:],
                  start=(d == 0), stop=(d == 3))
    m2 = pool.tile([1, HW], F32, tag="lnrow2", bufs=2, name=f"m2_{nm}")
    v.tensor_tensor(m2[:], mrow[:], mrow[:], AO.mult)
    vrow = pool.tile([1, HW], F32, tag="lnrow2", bufs=2, name=f"vr_{nm}")
    v.scalar_tensor_tensor(vrow[:], ps_q[0:1, 0:HW], 1.0 / 512.0, m2[:],
                           AO.mult, AO.subtract)
    srow = pool.tile([1, HW], F32, tag="lnrow2", bufs=2, name=f"sr_{nm}")
    sc.activation(srow[:], vrow[:], Act.Sqrt, bias=epsT[:])
    rrow = pool.tile([1, HW], F32, tag="lnrow2", bufs=2, name=f"rr_{nm}")
    v.reciprocal(rrow[:], srow[:])
    rrow_r = pool.tile([1, HW], F32R, tag="lnrow", bufs=2, name=f"rrr_{nm}")
    v.tensor_copy(rrow_r[:], rrow[:])
    ps_m = psum(f"lnm_{nm}")
    te.matmul(ps_m[:, 0:HW], ones1[:], mrow[:], start=True, stop=True)
    ps_r = psum(f"lnr_{nm}")
    te.matmul(ps_r[:, 0:HW], ones1[:], rrow_r[:], start=True, stop=True)
    for d in range(4):
        t1 = pool.tile([128, HW], F32, tag="lnt1", bufs=2, name=f"t1_{nm}_{d}")
        v.scalar_tensor_tensor(t1[:], src_tiles[d][:, hs], 1.0, ps_m[:, 0:HW],
                               AO.bypass, AO.subtract)
        t2 = pool.tile([128, HW], F32, tag="lnt2", bufs=2, name=f"t2_{nm}_{d}")
        v.tensor_tensor(t2[:], t1[:], ps_r[:, 0:HW], AO.mult)
        v.tensor_scalar(out_aps[d], t2[:], g_tab[:, d:d + 1], b_tab[:, d:d + 1],
                        AO.mult, AO.add)


def _layer(nc, tc, ap_, psum, live, l, b, xb,
           wq_sb, wk_sb, wv_sb, wo_sb, w1_sb, w2_sb, bv_sb,
           tabs, mw_sb, sxp, win, e8_sb, ones128, ones1, epsT, ones8=None):
    v, sc, te, sy = nc.vector, nc.scalar, nc.tensor, nc.sync
    AO = AluOpType
    t_bq, t_bk = tabs["tab_bq"], tabs["tab_bk"]
    t_bo, t_b2, t_b1 = tabs["tab_bo"], tabs["tab_b2"], tabs["tab_b1"]
    t_g1, t_o1 = tabs["tab_g1"], tabs["tab_o1"]
    t_g2, t_o2 = tabs["tab_g2"], tabs["tab_o2"]

    # ---- K^T (full width) and V+ (token-major) ----
    kT = [ap_.tile([128, L], F32R, tag="kT", bufs=4, name=f"kT{l}{b}_{d}")
          for d in range(4)]
    for hf in range(2):
        hs = slice(hf * HW, (hf + 1) * HW)
        for d in range(4):
            pk = psum(f"pk{l}{b}{hf}{d}")
            for k in range(4):
                te.matmul(pk[:, 0:HW], wk_sb[k][:, d * 128:(d + 1) * 128],
                          xb[k][:, hs], start=(k == 0), stop=(k == 3))
            sc.activation(kT[d][:, hs], pk[:, 0:HW], Act.Identity,
                          bias=t_bk[:, l * 4 + d:l * 4 + d + 1])
    vplus = []
    for t in range(6):
        a, bb = KT[t]
        w = bb - a
        vt = ap_.tile([128, 520], F32R, tag="vplus", bufs=6, name=f"vp{l}{b}_{t}")
        pv = psum(f"pv{l}{b}{t}")
        for k in range(4):
            te.matmul(pv[:w, 0:DM], xb[k][:, a:bb], wv_sb[k][:],
                      start=(k == 0), stop=(k == 3))
        ov = vt[:w].rearrange("p (h j) -> p h j", h=8)[:, :, 0:64]
        pvv = pv[:w, 0:DM].rearrange("p (h j) -> p h j", h=8)
        bvv = bv_sb[:w].rearrange("p (h j) -> p h j", h=8)
        v.tensor_tensor(ov, pvv, bvv, AO.add)
        nc.gpsimd.tensor_copy(vt[:w].rearrange("p (h j) -> p h j", h=8)[:, :, 64:65],
                               ones8[:w].unsqueeze(2))
        vplus.append(vt)

    for hf in range(2):
        hs = slice(hf * HW, (hf + 1) * HW)
        # ---- Q^T for this half ----
        qTh = [ap_.tile([128, HW], F32R, tag="qTh", bufs=4, name=f"qT{l}{b}{hf}_{d}")
               for d in range(4)]
        for d in range(4):
            pq = psum(f"pq{l}{b}{hf}{d}")
            for k in range(4):
                te.matmul(pq[:, 0:HW], wq_sb[k][:, d * 128:(d + 1) * 128],
                          xb[k][:, hs], start=(k == 0), stop=(k == 3))
            sc.activation(qTh[d][:, :], pq[:, 0:HW], Act.Identity,
                          bias=t_bq[:, l * 4 + d:l * 4 + d + 1])

        rs = [r for r in range(6) if live[(r, hf)]]
        oTh = [ap_.tile([128, HW], F32R, tag="oTh", bufs=4, name=f"oT{l}{b}{hf}_{d}")
               for d in range(4)]
        dnm = ap_.tile([8, HW], F32, tag="dnm", bufs=2, name=f"dn{l}{b}{hf}")
        for h in range(NH):
            d4, r64 = h // 2, (h % 2) * 64
            po = psum(f"po{l}{b}{hf}{h}")
            for ri, r in enumerate(rs):
                a, bb = KT[r]
                kp = bb - a
                ps_ = psum(f"ps{l}{b}{hf}{h}{r}")
                te.matmul(ps_[:kp, 0:HW], kT[d4][r64:r64 + 64, a:bb],
                          qTh[d4][r64:r64 + 64, :], start=True, stop=True)
                se = sxp[(r, hf)]
                for (q0, w, off) in win[(r, hf)]:
                    v.tensor_tensor(se[:kp, q0:q0 + w], ps_[:kp, q0:q0 + w],
                                    mw_sb[:kp, off:off + w], AO.add)
                    sc.activation(se[:kp, q0:q0 + w], se[:kp, q0:q0 + w],
                                  Act.Exp)
                te.matmul(po[0:65, 0:HW], vplus[r][:kp, h * 65:h * 65 + 65],
                          se[:kp, 0:HW], start=(ri == 0), stop=(ri == len(rs) - 1))
            v.tensor_copy(oTh[d4][r64:r64 + 64, :], po[0:64, 0:HW])
            dstage = ap_.tile([1, HW], F32, tag="dstage", bufs=2,
                              name=f"dg{l}{b}{hf}{h}")
            v.tensor_copy(dstage[:], po[64:65, 0:HW])
            sy.dma_start(dnm[h:h + 1, :], dstage[:])
        # ---- normalize O by softmax denominators ----
        dnr = ap_.tile([8, HW], F32, tag="dnm", bufs=2, name=f"dr{l}{b}{hf}")
        v.reciprocal(dnr[:], dnm[:])
        dnr_r = ap_.tile([8, HW], F32R, tag="dnmr", bufs=1, name=f"drr{l}{b}{hf}")
        v.tensor_copy(dnr_r[:], dnr[:])
        for m in range(4):
            prb = psum(f"prb{l}{b}{hf}{m}")
            te.matmul(prb[:, 0:HW], e8_sb[:, m * 128:(m + 1) * 128], dnr_r[:],
                      start=True, stop=True)
            v.tensor_tensor(oTh[m][:], oTh[m][:], prb[:, 0:HW], AO.mult)
        # ---- Wo proj + residual -> LN1 -> x ----
        resid = [ap_.tile([128, HW], F32R, tag="resid", bufs=4,
                          name=f"rs{l}{b}{hf}_{d}") for d in range(4)]
        for d in range(4):
            pa = psum(f"pa{l}{b}{hf}{d}")
            for k in range(4):
                te.matmul(pa[:, 0:HW], wo_sb[k][:, d * 128:(d + 1) * 128],
                          oTh[k][:], start=(k == 0), stop=(k == 3))
            v.scalar_tensor_tensor(resid[d][:], pa[:, 0:HW],
                                   t_bo[:, l * 4 + d:l * 4 + d + 1],
                                   xb[d][:, hs], AO.add, AO.add)
        _part_ln(nc, ap_, psum, resid, 0,
                 t_g1[:, l * 4:l * 4 + 4], t_o1[:, l * 4:l * 4 + 4],
                 [xb[d][:, hs] for d in range(4)],
                 ones128, ones1, epsT, f"l1_{l}{b}{hf}")
        # ---- FFN ----
        py = [psum(f"py{l}{b}{hf}{d}") for d in range(4)]
        for m in range(16):
            ph = psum(f"ph{l}{b}{hf}{m}")
            for k in range(4):
                te.matmul(ph[:, 0:HW], w1_sb[k][:, m * 128:(m + 1) * 128],
                          xb[k][:, hs], start=(k == 0), stop=(k == 3))
            hT = ap_.tile([128, HW], F32R, tag="hT", bufs=2, name=f"h{l}{b}{hf}{m}")
            sc.activation(hT[:], ph[:, 0:HW], Act.Gelu,
                          bias=t_b1[:, l * 16 + m:l * 16 + m + 1])
            for d in range(4):
                te.matmul(py[d][:, 0:HW], w2_sb[m][:, d * 128:(d + 1) * 128],
                          hT[:], start=(m == 0), stop=(m == 15))
        resid2 = [ap_.tile([128, HW], F32R, tag="resid", bufs=4,
                           name=f"r2{l}{b}{hf}_{d}") for d in range(4)]
        for d in range(4):
            v.scalar_tensor_tensor(resid2[d][:], py[d][:, 0:HW],
                                   t_b2[:, l * 4 + d:l * 4 + d + 1],
                                   xb[d][:, hs], AO.add, AO.add)
        _part_ln(nc, ap_, psum, resid2, 0,
                 t_g2[:, l * 4:l * 4 + 4], t_o2[:, l * 4:l * 4 + 4],
                 [xb[d][:, hs] for d in range(4)],
                 ones128, ones1, epsT, f"l2_{l}{b}{hf}")


# ======================= host side =======================
_PROG = None


def _pos_embed(n, d):
    pos = np.arange(n, dtype=np.float32)[:, None]
    div = np.exp(np.arange(0, d, 2, dtype=np.float32) * (-np.log(10000.0) / d))
    pe = np.zeros((n, d), dtype=np.float32)
    pe[:, 0::2] = np.sin(pos * div)
    pe[:, 1::2] = np.cos(pos * div)
    return pe


def _padtab(a, rows=128):
    # a: [n, cols] -> [128, cols] zero-padded
    out = np.zeros((rows, a.shape[1]), np.float32)
    out[:a.shape[0]] = a
    return out


def kernel(**inputs):
    global _PROG
    inputs = {k: np.asarray(v) for k, v in inputs.items()}
    attn_mask = inputs["attn_mask"]
    live = _nonempty(attn_mask)
    win, totw = _windows(attn_mask)
    if _PROG is None:
        _PROG = _build_program(live, win, totw)
    nc = _PROG

    x_enc = inputs["x_enc"].astype(np.float32)
    x_mark = inputs["x_mark_enc"].astype(np.float32)
    tok = inputs["tok_kernel"].astype(np.float32)

    X25 = np.concatenate([np.roll(x_enc, 1, axis=1), x_enc,
                          np.roll(x_enc, -1, axis=1), x_mark], axis=2)  # [B,512,25]
    X25T = np.ascontiguousarray(X25.transpose(0, 2, 1))                 # [B,25,512]
    W25 = np.concatenate([tok[0], tok[1], tok[2], inputs["mark_W"]], axis=0)
    Cemb = _pos_embed(SEQ, DM) + inputs["mark_b"]                       # [512,512]
    CembT = np.ascontiguousarray(Cemb.T).reshape(4, 128, SEQ)

    biasT = _mask_bias(attn_mask)  # [680, 680] (k, q) == transpose (symmetric)
    maskw = np.zeros((128, max(totw, 1)), np.float32)
    for (r, hf), lst in win.items():
        a, bb = KT[r]
        for (q0, w, off) in lst:
            maskw[:bb - a, off:off + w] = biasT[a:bb, hf * HW + q0:hf * HW + q0 + w]

    e8 = np.zeros((8, DM), np.float32)
    for h in range(8):
        e8[h, h * 64:(h + 1) * 64] = 1.0

    com = dict(
        w25=W25, cembT=CembT,
        downw=np.ascontiguousarray(inputs["down_W"].reshape(4, 128, 128)),
        convk=inputs["conv_K"].astype(np.float32),
        upw=inputs["up_W"].astype(np.float32),
        maskw=maskw, zeros_d=np.zeros((128, HW), np.float32), e8=e8,
        wq=inputs["Wq"] / 8.0, wk=inputs["Wk"], wv=inputs["Wv"], wo=inputs["Wo"],
        w1=inputs["W1"], w2=inputs["W2"],
        bvB=np.broadcast_to(inputs["bv"][:, None, :], (NL, 128, DM)).copy(),
        tab_bq=(inputs["bq"] / 8.0).reshape(NL * 4, 128).T.copy(),
        tab_bk=inputs["bk"].reshape(NL * 4, 128).T.copy(),
        tab_bo=inputs["bo"].reshape(NL * 4, 128).T.copy(),
        tab_b2=inputs["b2"].reshape(NL * 4, 128).T.copy(),
        tab_b1=inputs["b1"].reshape(NL * 16, 128).T.copy(),
        tab_g1=inputs["ln1_g"].reshape(NL * 4, 128).T.copy(),
        tab_o1=inputs["ln1_b"].reshape(NL * 4, 128).T.copy(),
        tab_g2=inputs["ln2_g"].reshape(NL * 4, 128).T.copy(),
        tab_o2=inputs["ln2_b"].reshape(NL * 4, 128).T.copy(),
        ones128_d=np.ones((128, 1), np.float32),
        ones1_d=np.ones((1, 128), np.float32),
        ones8_d=np.ones((128, 8), np.float32),
        tab_cn=np.concatenate([inputs["cn_g"].reshape(4, 128).T,
                               inputs["cn_b"].reshape(4, 128).T], axis=1).copy(),
    )
    cs = np.zeros((128, 16), np.float32)
    cs[:, 0] = inputs["down_b"]
    for i in range(3):
        cs[:, 1 + i] = inputs["bn_g"][i]
        cs[:, 4 + i] = inputs["conv_b"][i]
        cs[:, 7 + i] = inputs["bn_b"][i]
    cs[:, 12:16] = inputs["up_b"].reshape(4, 128).T
    com["tab_cs"] = cs
    com = {k: np.ascontiguousarray(v, np.float32) for k, v in com.items()}

    in_maps = []
    for c in range(N_CORES):
        order = [2 * c, 2 * c + 1] + [i for i in range(NB) if i not in (2 * c, 2 * c + 1)]
        m = dict(com)
        m["x25t"] = np.ascontiguousarray(X25T[order])
        in_maps.append(m)

    res = run_bass_kernel_spmd(nc, in_maps, core_ids=list(range(N_CORES)))

    # assemble: out per core [2, 4, 128, 680] feature-major -> [B, 680, 512]
    X = np.empty((NB, L, DM), np.float32)
    for c in range(N_CORES):
        o = res.results[c]["out"]  # [2, 4, 128, 680]
        for j in range(LB):
            X[2 * c + j] = o[j].reshape(DM, L).T
    gidx = np.asarray(inputs["gather_idx"]).astype(np.int64)
    out = X[:, gidx, :].reshape(NB, SEQ, NH * 4 * DKH)
    return out.astype(np.float32)
